# revision 89
# baseline (speedup 1.0000x reference)
"""Trainium2 Bass kernel for nn_CBAMSpaceMask (CBAM spatial mask over T timestep blocks).

Math per timestep block t (3 channels):
  mx_c = maxpool3x3(x_c)          (stride 1, -inf pad == replicate pad)
  av_c = avgpool3x3(x_c)/9        (zero pad, count_include_pad)
  y_t  = sum_c wM_c * mx_c + wA_c * av_c + b   (3x3 conv, zero pad)
  out[3t+c] = sigmoid(leakyrelu(y_t))          (broadcast over c)

Design (per core = 1 batch element, pure data parallel over batch):
  - host-side re-layout (pure layout, no compute): x4[g, r, j, c, w] =
    xpad[r+j, 6g+c, w] bf16 materializes the vertical-shift triplet
    contiguously per (group, row), so every main input load is ONE
    software-DGE trigger per group with fully-contiguous 18KB-per-partition
    runs on both sides (~35% faster DMA than fragmented reads); xt2 is the
    same idea for the small last-8-rows loads
  - groups of 6 planes (= 2 timesteps, one matmul pair); both row-subs
    (y rows 0:124 and 124:248) share a [128, 2sub, 3shift, 6, W] tile
  - pools: vertical 3-row max (2 DVE ops over the shift slices), horizontal
    3-tap max and box sum (2 DVE ops each) into 258-col zero-padded mx/bh;
    vertical box sum of the avg path folded into the conv operator (op@Bv);
    pad-column zeroing via ONE strided-AP memset per tile (cols 0 and 257)
  - conv: banded-Toeplitz matmuls on PE; timestep-paired rhs (planes
    {c, c+3} via stride-3 slice) -> all matmuls full-width N=512, padded
    windows supply the conv zero padding; psum [124, 2, 256]; max-path
    matmuls issued first so the bh pools may lag the mx pools
  - MOVED groups: the avg path runs on PE as 5 horizontal taps directly
    against X (effective kernel w_avg*[1,1,1], vertical box still in the
    band), with clipped rhs windows accumulating into column-offset psum
    slices (unwritten psum columns realize the conv zero pad) -> their bh
    DVE pools are skipped entirely. k=1 balances DVE (the bottleneck
    engine) against PE's late-phase wall.
  - last-8-rows chunk: rows 246..255 packed per quadrant at partition
    bases 0/32/64/96; kw taps folded into K=30 stacked matmuls using R/L
    column-shifted copies (shifts carry the zero edge in from memset pad
    cols); c2 pairs run at iterations 3..6, never in a low-p-state tail
  - epilogue: ACT Prelu(psum+bias, alpha=.01) IN PLACE on psum (same ACT
    table as sigmoid -> no table reloads) -> ACT Sigmoid -> bf16 sg tile.
    Output is ONE channel per timestep ([T, H, W]); the host broadcasts to
    the 3 channels (reference broadcasts before the elementwise sigmoid, so
    results are identical) -> output DMA volume and sigmoid work cut 3x
  - c2 sigmoids write a persistent [8, 16, W] tile; ONE final DMA stores
    rows 248..255 for all timesteps
  - startup: all early DMAs share the gpsimd software-DGE FIFO so issue
    order is true bandwidth priority (g0 per-sub, cst chunks split by
    first-use mat ranges, t2, g1, g2); group 0 is pooled per-sub with
    dedicated tiles so PE starts ~17us in; a burst of consumer-less
    keepalive matmuls into a scratch psum bank holds the PE p-state up
    through the one unavoidable pipeline-fill gap; loads run 3 groups
    ahead, pools 2 ahead of convs, output triggers age 2 checkpoints so
    no gpsimd trigger ever blocks on an incomplete producer
"""
import sys

sys.path.insert(0, "/opt/trn_rl_repo")

import numpy as np
import ml_dtypes
from contextlib import ExitStack

import concourse.bass as bass
import concourse.tile as tile
from concourse import bacc, mybir
from concourse.bass_utils import run_bass_kernel_spmd

F32 = mybir.dt.float32
BF16 = mybir.dt.bfloat16

B, CTOT, H, W = 8, 48, 256, 256
T = 16
N_CORES = 8
NGRP = 8            # groups of 6 planes = 2 timesteps
GP = 6              # planes per group
# main chunk geometry: y rows [m0,m1) from x rows [r0,r1)
SUBS = [(0, 124, 0, 128), (124, 248, 122, 250)]
C2 = (248, 256, 246, 256)   # last-8-rows chunk
NMAIN = 2 * 3 * 3 * 2       # path, c, kw, sub
NC2 = 2 * 3                 # path, c (kw folded into K=30)
NAVG5 = 3 * 5 * 2           # c, s, sub (avg path as 5-tap on X, no bh pools)
NCORR = 3 * 2 * 2           # c, side, sub (avg5 edge-column corrections)
NMAT = NMAIN + NC2 + NAVG5 + NCORR
# groups whose avg path runs on PE as 5 horizontal taps against X directly
# (skipping the bh DVE pools) — balances DVE (bottleneck) against PE slack
MOVED = (1,)
AVG5_TAPS = (-2, -1, 1, 2, 0)   # s=0 last: the stop matmul is full width
QP_T2 = 12                      # planes per t2 quadrant (2 groups)

_cache = {}


def _build_stack(conv_w):
    """lhsT stack [128, NMAT, 128] bf16.

    mats 0..35: main-sub ops, idx = ((path*3 + c)*3 + kw)*2 + sub,
      lhsT = op[m0:m1, r0:r1].T  ([K=128, M=124])
    mats 36..41: chunk-2 stacked ops, idx = 36 + path*3 + c,
      [K=30, M=8]: K blocks of 10 rows for kw = 1 (center), 0, 2,
      replicated at partition bases 0/32/64/96.
    """
    w = conv_w[0].astype(np.float64)  # [6, 3, 3]
    Bv = np.zeros((H, H))
    for i in (-1, 0, 1):
        Bv += np.eye(H, k=i)
    stack = np.zeros((128, NMAT, 128), dtype=np.float64)

    def band_op(path, c, kw):
        op = np.zeros((H, H))
        k2d = w[2 * c] if path == 0 else w[2 * c + 1]
        for kh in range(3):
            op += k2d[kh, kw] * np.eye(H, k=kh - 1)
        if path == 1:
            op = (op @ Bv) / 9.0
        return op

    def band_op5(c, s):
        """Avg path folded horizontally: 5-tap effective kernel e[s] =
        (w_avg[kh, :] * [1,1,1])[s]; vertical box stays in the band."""
        op = np.zeros((H, H))
        k2d = w[2 * c + 1]
        for kh in range(3):
            e = 0.0
            for kw in range(3):
                if abs(kw - 1 - s) <= 1:
                    e += k2d[kh, kw]
            op += e * np.eye(H, k=kh - 1)
        return (op @ Bv) / 9.0

    # mat layout is ordered by first use so the cst load can be split into
    # chunks positioned in the early DMA FIFO:
    #   [0:18]  sub0 main (max+avg3)   — first matmuls of group 0 sub 0
    #   [18:36] sub1 main
    #   [36:66] avg5 (MOVED groups)    — first used by group 1
    #   [66:72] c2                     — first used around iteration 3
    for path in range(2):
        for c in range(3):
            for kw in range(3):
                op = band_op(path, c, kw)
                for sub, (m0, m1, r0, r1) in enumerate(SUBS):
                    mat = _mat_main(path, c, kw, sub)
                    lhsT = op[m0:m1, r0:r1].T  # [K, M]
                    K, M = lhsT.shape
                    stack[:K, mat, :M] = lhsT
            # chunk 2: kw-stacked [30, 8]
            mat = _mat_c2(path, c)
            m0, m1, r0, r1 = C2
            for kwi, kw in enumerate((1, 0, 2)):
                lhsT = band_op(path, c, kw)[m0:m1, r0:r1].T  # [10, 8]
                for base in (0, 32, 64, 96):
                    stack[base + 10 * kwi:base + 10 * kwi + 10, mat, :8] = lhsT
    for c in range(3):
        for si, s in enumerate(AVG5_TAPS):
            op = band_op5(c, s)
            for sub, (m0, m1, r0, r1) in enumerate(SUBS):
                mat = _mat_avg5(c, si, sub)
                lhsT = op[m0:m1, r0:r1].T
                K, M = lhsT.shape
                stack[:K, mat, :M] = lhsT
    # avg5 edge corrections: the horizontal fold e = w * [1,1,1] is the
    # UNclipped composition, so at image cols 0/255 it spuriously includes
    # the kw=0/kw=2 weight against the edge x column (the reference zeroes
    # the whole out-of-range pooled value). Subtract that term.
    for c in range(3):
        for side, kw in ((0, 0), (1, 2)):
            op = np.zeros((H, H))
            for kh in range(3):
                op += w[2 * c + 1][kh, kw] * np.eye(H, k=kh - 1)
            op = -(op @ Bv) / 9.0
            for sub, (m0, m1, r0, r1) in enumerate(SUBS):
                lhsT = op[m0:m1, r0:r1].T
                K, M = lhsT.shape
                stack[:K, _mat_corr(c, side, sub), :M] = lhsT
    return stack.astype(ml_dtypes.bfloat16)


def _mat_main(path, c, kw, sub):
    return sub * 18 + (path * 3 + c) * 3 + kw


def _mat_avg5(c, si, sub):
    return 36 + sub * 15 + c * 5 + si


def _mat_c2(path, c):
    return 66 + path * 3 + c


def _mat_corr(c, side, sub):
    return 72 + (c * 2 + side) * 2 + sub


def _prep_x(xi):
    """Host-side re-layout.

    xp [row+pad, plane, w] bf16: row r holds image row r-1; rows 0 and 257
    replicate the image edge rows (the maxpool clamp; conv coefficients
    there are zero). Used by the small t2 (last-8-rows) loads.

    x4 [group, row, shift, plane-in-group, w] bf16: x4[g, r, j] = xp[r+j]
    for the 6 planes of group g. The shift triplet a partition needs for
    the vertical 3-max is CONTIGUOUS per (g, r), so every main input load
    is a fully-contiguous 18KB-per-partition HBM read (the DMA engines run
    ~35% faster on unfragmented source runs).
    """
    xp = np.empty((H + 2, CTOT, W), dtype=ml_dtypes.bfloat16)
    xp[1:H + 1] = xi.transpose(1, 0, 2)
    xp[0] = xp[1]
    xp[H + 1] = xp[H]
    v = np.lib.stride_tricks.as_strided(
        xp, shape=(NGRP, H, 3, GP, W),
        strides=(GP * W * 2, CTOT * W * 2, CTOT * W * 2, W * 2, 2))
    # t2 feed: xt2[q, j, r, c, w] = xp[246 + r + j, 12q + c, w] — contiguous
    # per (q, j, r) so the 12 small t2 loads are unfragmented
    r0 = C2[2]
    v2 = np.lib.stride_tricks.as_strided(
        xp[r0:], shape=(4, 3, 10, 12, W),
        strides=(12 * W * 2, CTOT * W * 2, CTOT * W * 2, W * 2, 2))
    return {"x4": np.ascontiguousarray(v), "xt2": np.ascontiguousarray(v2)}


def _build_program():
    nc = bacc.Bacc("TRN2", target_bir_lowering=False, debug=False, enable_asserts=False)
    x4_ap = nc.dram_tensor("x4", [NGRP, H, 3, GP, W], BF16,
                           kind="ExternalInput").ap()
    xt2_ap = nc.dram_tensor("xt2", [4, 3, 10, QP_T2, W], BF16,
                            kind="ExternalInput").ap()
    cst_ap = nc.dram_tensor("cst", [128, NMAT, 128], BF16, kind="ExternalInput").ap()
    bias_ap = nc.dram_tensor("bias", [128, 1], F32, kind="ExternalInput").ap()
    # bf16, one channel per timestep: sigmoid outputs lie in (0,1) so bf16
    # quantization (~0.4% rel) is far inside the accuracy budget; the host
    # upcasts to f32 and broadcasts each timestep mask to its 3 channels.
    out_ap = nc.dram_tensor("out", [T, H, W], BF16, kind="ExternalOutput").ap()

    MAXOP = mybir.AluOpType.max
    ADDOP = mybir.AluOpType.add
    RWST = CTOT * W          # HBM row stride (elements)

    with tile.TileContext(nc) as tc, ExitStack() as ctx:
        const_pool = ctx.enter_context(tc.tile_pool(name="const", bufs=1))
        psum_pool = ctx.enter_context(tc.tile_pool(name="psum", bufs=6, space="PSUM"))
        warm_pool = ctx.enter_context(tc.tile_pool(name="warm", bufs=1, space="PSUM"))
        sg_pool = ctx.enter_context(tc.tile_pool(name="sg", bufs=6))
        t2_pool = ctx.enter_context(tc.tile_pool(name="t2", bufs=1))
        # 4 x bufs: load_xud(g+3) at iteration g then recycles the buffer of
        # group g-1, whose conv readers (MOVED groups read X as the avg-path
        # rhs) were issued at iteration g-1 — program order stays consistent
        x_pool = ctx.enter_context(tc.tile_pool(name="xload", bufs=4))
        mxbh_pool = ctx.enter_context(tc.tile_pool(name="mxbh", bufs=3))
        g0_pool = ctx.enter_context(tc.tile_pool(name="g0", bufs=1))

        cst = const_pool.tile([128, NMAT, 128], BF16, tag="cst")
        bias = const_pool.tile([128, 1], F32, tag="bias")
        # (cst/bias DMAs are issued AFTER the first input loads: cst is only
        # needed by the first matmul ~13us in, while the input loads gate the
        # DVE pools — the 1.4MB cst transfer must not hog the DMA engines
        # during the first microseconds)

        # ---- t2 tiles: rows 246..255 of quadrant q (planes 12q..12q+11) at
        # partitions 32q..32q+9. MX/BH are padded to 258 cols (data at cols
        # 1..256, zero pads) and also hold R/L column-shifted copies at
        # partition offsets +10 / +20 (kw-folded K=30).
        WP = W + 2
        QP = 12  # planes per t2 quadrant (2 groups)
        T2X = t2_pool.tile([128, QP, W], BF16, tag="t2x")
        T2U = t2_pool.tile([128, QP, W], BF16, tag="t2u")
        T2D = t2_pool.tile([128, QP, W], BF16, tag="t2d")
        T2MX = t2_pool.tile([128, QP, WP], BF16, tag="t2mx")
        T2BH = t2_pool.tile([128, QP, WP], BF16, tag="t2bh")
        # c2 sigmoid accumulator: rows 248..255 x all 16 timesteps; ONE
        # final DMA stores it
        C2OUT = t2_pool.tile([8, T, W], BF16, tag="c2out")
        # (no full-tile zeroing: garbage in gap partitions only flows into
        # regions later overwritten by the shift DMAs or never read; the
        # pad columns that ARE read get strided memsets in t2_pools)

        # ---- fused input load: tile [128, 2 sub, 3 shift, GP, W]; shift j
        # holds padded rows (r0_sub + p + j) so the vertical 3-max is three
        # aligned slices of ONE tile. The host-materialized x4 layout makes
        # the triplet contiguous per (group, row): ONE trigger per group,
        # fully-contiguous 18KB packets on both sides.
        xud_tiles = {}
        RB = 3 * GP * W  # x4 row block (one partition's triplet), elements

        def _src_ap(g, subs=(0, 1)):
            dims = [[RB, 128]]
            if len(subs) == 2:
                dims.append([SUBS[1][2] * RB, 2])
            dims += [[1, RB]]
            off = g * H * RB + SUBS[subs[0]][2] * RB
            return bass.AP(x4_ap.tensor, off, dims)

        def load_xud(g):
            X = x_pool.tile([128, 2, 3, GP, W], BF16, tag="x")
            nc.gpsimd.dma_start(out=X[:], in_=_src_ap(g))
            xud_tiles[g] = X
            return X

        def load_xc(g):
            """Center slice only (both subs) — feeds a MOVED group's avg5
            matmuls, which can then run before the U/D halves even arrive."""
            X = x_pool.tile([128, 2, 3, GP, W], BF16, tag="x")
            dims = [[RB, 128], [SUBS[1][2] * RB, 2], [1, GP * W]]
            nc.gpsimd.dma_start(out=X[:, :, 1:2],
                                in_=bass.AP(x4_ap.tensor,
                                            g * H * RB + GP * W, dims))
            xud_tiles[g] = X
            return X

        def load_ud(g):
            X = xud_tiles[g]
            for sub in range(2):
                off = g * H * RB + SUBS[sub][2] * RB
                dims = [[RB, 128], [2 * GP * W, 2], [1, GP * W]]
                nc.gpsimd.dma_start(out=X[:, sub, 0:3:2],
                                    in_=bass.AP(x4_ap.tensor, off, dims))

        def pools(g, X=None, mx=None, bh=None, sub=None):
            """DVE pools; when sub is given, operate on that sub slice only
            (used for group 0's fast start with dedicated tiles). Groups in
            MOVED skip the bh pools (their avg path runs on PE against X)."""
            moved = g in MOVED
            if X is None:
                X = xud_tiles.pop(g)
            if mx is None:
                mx = mxbh_pool.tile([128, 2, GP, WP], BF16, tag="mx")
                bh = None if moved else mxbh_pool.tile([128, 2, GP, WP], BF16,
                                                       tag="bh")
            s = slice(None) if sub is None else slice(sub, sub + 1)
            D, XC, U = X[:, s, 0], X[:, s, 1], X[:, s, 2]
            # one strided memset zeroes both pad columns (0 and 257)
            nc.vector.memset(mx[:, s, :, 0:258:257], 0)
            # vertical 3-row max (DVE), in place into the U slice
            vx = U
            nc.vector.tensor_tensor(out=vx, in0=U, in1=D, op=MAXOP)
            nc.vector.tensor_tensor(out=vx, in0=vx, in1=XC, op=MAXOP)
            # horizontal 3-tap max (DVE) into padded mx
            nc.vector.tensor_tensor(out=mx[:, s, :, 1:256], in0=vx[:, :, :, 0:255],
                                    in1=vx[:, :, :, 1:256], op=MAXOP)
            nc.vector.tensor_copy(mx[:, s, :, 256:257], vx[:, :, :, 255:256])
            nc.vector.tensor_tensor(out=mx[:, s, :, 2:257], in0=mx[:, s, :, 2:257],
                                    in1=vx[:, :, :, 0:255], op=MAXOP)
            if not moved:
                # horizontal 3-tap box sum (DVE) into padded bh
                nc.vector.memset(bh[:, s, :, 0:258:257], 0)
                nc.vector.tensor_tensor(out=bh[:, s, :, 1:256],
                                        in0=XC[:, :, :, 0:255],
                                        in1=XC[:, :, :, 1:256], op=ADDOP)
                nc.vector.tensor_copy(bh[:, s, :, 256:257], XC[:, :, :, 255:256])
                nc.vector.tensor_tensor(out=bh[:, s, :, 2:257],
                                        in0=bh[:, s, :, 2:257],
                                        in1=XC[:, :, :, 0:255], op=ADDOP)
            return mx, bh, X

        def load_t2(q):
            # gpsimd FIFO (so these cannot steal engine bandwidth from the
            # earlier critical loads); the xt2 layout keeps them contiguous
            b = 32 * q
            nc.gpsimd.dma_start(out=T2X[b:b + 10], in_=xt2_ap[q, 1])
            nc.gpsimd.dma_start(out=T2U[b:b + 10], in_=xt2_ap[q, 2])
            nc.gpsimd.dma_start(out=T2D[b:b + 10], in_=xt2_ap[q, 0])

        def _t2_shifts(tl):
            # R/L column-shifted copies into partition blocks +10 / +20.
            # All matmul rhs windows read cols 1..256 of their block:
            #   block +10 pairs kw=0 (needs P[w-1]): dest col j <- data col j-1
            #   block +20 pairs kw=2 (needs P[w+1]): dest col j <- data col j+1
            # The widened [0:256]/[2:258] sources carry the zero pad edge.
            for q in range(4):
                b = 32 * q
                nc.sync.dma_start(out=tl[b + 10:b + 20, :, 1:257],
                                  in_=tl[b:b + 10, :, 0:256])
                nc.sync.dma_start(out=tl[b + 20:b + 30, :, 1:257],
                                  in_=tl[b:b + 10, :, 2:258])

        def t2_pools(chunk):
            """Pools over the packed t2 tile, issued in 3 chunks so the DVE
            bursts interleave between main-group pools instead of stalling
            a whole group's worth of PE work."""
            vx = T2U
            if chunk == 0:
                nc.vector.tensor_tensor(out=vx[:], in0=T2U[:], in1=T2D[:],
                                        op=MAXOP)
                nc.vector.tensor_tensor(out=vx[:], in0=vx[:], in1=T2X[:],
                                        op=MAXOP)
                nc.vector.memset(T2MX[:, :, 0:258:257], 0)
                nc.vector.memset(T2BH[:, :, 0:258:257], 0)
            elif chunk == 1:
                nc.vector.tensor_tensor(out=T2MX[:, :, 1:256], in0=vx[:, :, 0:255],
                                        in1=vx[:, :, 1:256], op=MAXOP)
                nc.vector.tensor_copy(T2MX[:, :, 256:257], vx[:, :, 255:256])
                nc.vector.tensor_tensor(out=T2MX[:, :, 2:257],
                                        in0=T2MX[:, :, 2:257],
                                        in1=vx[:, :, 0:255], op=MAXOP)
                _t2_shifts(T2MX)
            else:
                nc.vector.tensor_tensor(out=T2BH[:, :, 1:256], in0=T2X[:, :, 0:255],
                                        in1=T2X[:, :, 1:256], op=ADDOP)
                nc.vector.tensor_copy(T2BH[:, :, 256:257], T2X[:, :, 255:256])
                nc.vector.tensor_tensor(out=T2BH[:, :, 2:257],
                                        in0=T2BH[:, :, 2:257],
                                        in1=T2X[:, :, 0:255], op=ADDOP)
                _t2_shifts(T2BH)

        out_ready = []   # sigmoids surely complete: safe to issue triggers
        out_recent = []  # freshly issued sigmoids: age one checkpoint first

        def epilogue_lrelu(ps, M):
            """ACT Prelu(psum + bias) IN PLACE on the psum bank. Prelu
            (parametric_relu) lives in the same ACT function table as
            sigmoid, so alternating them costs no ACT_TABLE_LOADs."""
            nc.scalar.activation(ps[0:M], ps[0:M],
                                 mybir.ActivationFunctionType.Prelu,
                                 bias=bias[0:M], scale=1.0, alpha=0.01)
            return ps

        def epilogue_sigmoid(ps, M, t0, m0, m1):
            """Sigmoid psum -> bf16 sg (one channel per timestep). Output
            DMA issue is DEFERRED (gpsimd queue is in-order: a trigger
            waiting on its sigmoid would stall later load triggers)."""
            sg = sg_pool.tile([128, 2, W], BF16, tag="epis")
            nc.scalar.activation(sg[0:M], ps[0:M],
                                 mybir.ActivationFunctionType.Sigmoid)
            out_recent.append((sg, M, t0, m0, m1))

        def flush_outputs(final=False):
            for sg, M, t0, m0, m1 in out_ready:
                dst = out_ap[t0:t0 + 2, m0:m1, :].transpose([1, 0, 2])
                nc.gpsimd.dma_start(out=dst, in_=sg[0:M])
            out_ready.clear()
            out_ready.extend(out_recent)
            out_recent.clear()
            if final and out_ready:
                flush_outputs()

        def conv_sub(g, sub, mx, bh, X):
            """Main-chunk accumulation for group g's timestep pair.

            Max-path matmuls are full-width N=512 against the padded mx
            (whose zero pads supply the conv zero padding). For MOVED
            groups the avg path is 5 horizontal taps against X directly:
            clipped rhs windows accumulate into column-offset psum slices,
            so unwritten psum columns realize the conv zero padding.
            """
            m0, m1, r0, r1 = SUBS[sub]
            M, K = m1 - m0, r1 - r0
            moved = g in MOVED
            sb = 0 if mx.shape[1] == 1 else sub
            ps = psum_pool.tile([128, 2, W], F32, tag="ps")
            i, n = 0, 30 if moved else 18
            for c in range(3):
                for kw in (1, 0, 2):
                    s = kw - 1
                    mat = _mat_main(0, c, kw, sub)
                    rhs = mx[0:K, sb, c:c + 4:3, 1 + s:257 + s]
                    nc.tensor.matmul(ps[0:M], cst[0:K, mat, 0:M], rhs,
                                     start=(i == 0), stop=(i == n - 1))
                    i += 1
            if moved:
                XC = X[:, sb, 1]
                for si, s in enumerate(AVG5_TAPS):
                    a, b = max(0, s), W + min(0, s)
                    for c in range(3):
                        mat = _mat_avg5(c, si, sub)
                        rhs = XC[0:K, c:c + 4:3, a:b]
                        nc.tensor.matmul(ps[0:M, :, a - s:b - s],
                                         cst[0:K, mat, 0:M], rhs,
                                         start=False, stop=(i == n - 1))
                        i += 1
                # edge-column corrections (N=2 each): subtract the spurious
                # kw=0 / kw=2 term at image cols 0 / 255
                for side, col in ((0, 0), (1, W - 1)):
                    for c in range(3):
                        rhs = XC[0:K, c:c + 4:3, col:col + 1]
                        nc.tensor.matmul(ps[0:M, :, col:col + 1],
                                         cst[0:K, _mat_corr(c, side, sub), 0:M],
                                         rhs, start=False, stop=(i == n - 1))
                        i += 1
            else:
                for c in range(3):
                    for kw in (1, 0, 2):
                        s = kw - 1
                        mat = _mat_main(1, c, kw, sub)
                        rhs = bh[0:K, sb, c:c + 4:3, 1 + s:257 + s]
                        nc.tensor.matmul(ps[0:M], cst[0:K, mat, 0:M], rhs,
                                         start=False, stop=(i == n - 1))
                        i += 1
            epilogue_lrelu(ps, M)
            return ps, M, 2 * g, m0, m1

        def conv_avg5(g, sub, X):
            """Open a MOVED group's psum with its 15 avg5 taps (needs only
            the XC slice + cst chunk C — runs while U/D are still loading).
            The full-width s=0 tap goes first to initialize the psum."""
            m0, m1, r0, r1 = SUBS[sub]
            M, K = m1 - m0, r1 - r0
            ps = psum_pool.tile([128, 2, W], F32, tag="ps")
            XC = X[:, sub, 1]
            first = True
            for si, s in ((4, 0), (0, -2), (1, -1), (2, 1), (3, 2)):
                a, b = max(0, s), W + min(0, s)
                for c in range(3):
                    rhs = XC[0:K, c:c + 4:3, a:b]
                    nc.tensor.matmul(ps[0:M, :, a - s:b - s],
                                     cst[0:K, _mat_avg5(c, si, sub), 0:M],
                                     rhs, start=first, stop=False)
                    first = False
            return ps

        def conv_max(g, sub, mx, ps):
            """Close the psum with the 9 max-path matmuls."""
            m0, m1, r0, r1 = SUBS[sub]
            M, K = m1 - m0, r1 - r0
            i = 0
            for c in range(3):
                for kw in (1, 0, 2):
                    s = kw - 1
                    rhs = mx[0:K, sub, c:c + 4:3, 1 + s:257 + s]
                    nc.tensor.matmul(ps[0:M], cst[0:K, _mat_main(0, c, kw, sub), 0:M],
                                     rhs, start=False, stop=(i == 8))
                    i += 1
            epilogue_lrelu(ps, M)
            return ps, M, 2 * g, m0, m1

        def conv_c2(g):
            """Last-8-rows accumulation (kw-folded, K=30) for group g; the
            sigmoid lands in the persistent C2OUT tile."""
            m0, m1, r0, r1 = C2
            M = m1 - m0
            b = 32 * (g // 2)
            pb = 6 * (g % 2)
            ps = psum_pool.tile([128, 2, W], F32, tag="ps")
            idx = 0
            for path in range(2):
                for c in range(3):
                    mat = _mat_c2(path, c)
                    src = T2MX if path == 0 else T2BH
                    rhs = src[b:b + 30, pb + c:pb + c + 4:3, 1:257]
                    nc.tensor.matmul(ps[0:M], cst[b:b + 30, mat, 0:M], rhs,
                                     start=(idx == 0), stop=(idx == NC2 - 1),
                                     tile_position=(b, 0))
                    idx += 1
            epilogue_lrelu(ps, M)
            nc.scalar.activation(C2OUT[0:M, 2 * g:2 * g + 2], ps[0:M],
                                 mybir.ActivationFunctionType.Sigmoid)

        # ---- schedule: group 0 is loaded per-sub with dedicated pool tiles
        # so the first matmul only waits on sub 0's load + 6 DVE ops. Later
        # loads run 3 groups ahead; pools for g+2 are issued at the top of
        # iteration g so they execute while PE runs group g's convs. Output
        # triggers age through two checkpoints before issue.
        # conv_c2(g) is deferred three iterations (it only needs the t2 pools
        # and a psum bank), so t2 pool work stays off the early critical path
        # ---- early DMA FIFO (all on the gpsimd queue, so issue order is
        # bandwidth priority): g0 per-sub first (sub0 gates everything),
        # then the cst chunks in first-use order, t2, g1, g2.
        g0X = x_pool.tile([128, 2, 3, GP, W], BF16, tag="x")
        for sub in range(2):
            nc.gpsimd.dma_start(out=g0X[:, sub:sub + 1], in_=_src_ap(0, (sub,)))
        nc.gpsimd.dma_start(out=cst[:, 0:18], in_=cst_ap[:, 0:18, :])
        nc.gpsimd.dma_start(out=cst[:, 18:36], in_=cst_ap[:, 18:36, :])
        for q in range(4):
            load_t2(q)
        load_xud(1)
        nc.gpsimd.dma_start(out=cst[:, 36:66], in_=cst_ap[:, 36:66, :])
        load_xud(2)
        nc.gpsimd.dma_start(out=cst[:, 66:84], in_=cst_ap[:, 66:84, :])
        nc.gpsimd.dma_start(out=bias[:], in_=bias_ap)
        # DVE: g0 pools (split per sub for the earliest first matmul), then
        # the t2 chunks woven between p1/p2
        g0t = []
        for sub in range(2):
            mxs = g0_pool.tile([128, 1, GP, WP], BF16, tag=f"g0mx{sub}")
            bhs = (None if 0 in MOVED else
                   g0_pool.tile([128, 1, GP, WP], BF16, tag=f"g0bh{sub}"))
            mxs, bhs, _ = pools(0, X=g0X[:, sub:sub + 1], mx=mxs, bh=bhs, sub=0)
            g0t.append((mxs, bhs, g0X[:, sub:sub + 1]))
        t2_pools(0)
        pools_of = {1: pools(1)}
        t2_pools(1)
        pools_of[2] = pools(2)
        t2_pools(2)
        # c2 pairs run at iterations 3..6, issued BEFORE the group's convs:
        # they fill the PE stall windows where pools lag, and none are left
        # for a low-p-state tail
        for g in range(NGRP):
            if g + 3 < NGRP:
                load_xud(g + 3)
            if g + 2 < NGRP and g >= 1:
                pools_of[g + 2] = pools(g + 2)
            if 3 <= g <= 6:
                conv_c2(2 * (g - 3))
                conv_c2(2 * (g - 3) + 1)
            if g == 0:
                epilogue_sigmoid(*conv_sub(0, 0, *g0t[0]))
                epilogue_sigmoid(*conv_sub(0, 1, *g0t[1]))
                # PE keepalive: group 1's matmuls cannot start until its
                # pools clear the t2 chunks (~18us idle). Idle drops the PE
                # to a low p-state and the first ~20 real matmuls after the
                # gap run ~2x slow. Dummy matmuls into a scratch psum bank
                # (no consumers) hold the clock up through the gap.
                scratch = warm_pool.tile([128, 2, W], F32, tag="warm")
                for i in range(44):
                    nc.tensor.matmul(scratch[0:124], cst[0:128, 0, 0:124],
                                     g0t[0][0][0:128, 0, 0:4:3, 1:257],
                                     start=(i == 0), stop=(i == 43))
            else:
                mx, bh, X = pools_of.pop(g)
                epilogue_sigmoid(*conv_sub(g, 0, mx, bh, X))
                epilogue_sigmoid(*conv_sub(g, 1, mx, bh, X))
            flush_outputs()
        flush_outputs(final=True)
        # one DMA for all last-8 rows: [T, 8, W] <- C2OUT[0:8] transposed
        m0 = C2[0]
        nc.sync.dma_start(out=out_ap[:, m0:m0 + 8, :].transpose([1, 0, 2]),
                          in_=C2OUT[0:8])

    nc.compile()
    return nc


def kernel(input_tensor, conv_w, conv_b):
    input_tensor = np.ascontiguousarray(np.asarray(input_tensor, dtype=np.float32))
    conv_w = np.asarray(conv_w, dtype=np.float32)
    conv_b = np.asarray(conv_b, dtype=np.float32)

    if "nc" not in _cache:
        _cache["nc"] = _build_program()
    nc = _cache["nc"]

    stack = _build_stack(conv_w)
    bias_vec = np.full((128, 1), conv_b[0], dtype=np.float32)
    in_maps = [
        {**_prep_x(input_tensor[i]), "cst": stack, "bias": bias_vec}
        for i in range(N_CORES)
    ]
    res = run_bass_kernel_spmd(nc, in_maps, list(range(N_CORES)))
    # [T, H, W] bf16 per core -> broadcast each timestep mask to 3 channels
    out = np.stack([res.results[i]["out"] for i in range(N_CORES)], axis=0)
    out = np.repeat(out.astype(np.float32), 3, axis=1)
    return out


if __name__ == "__main__":
    rng = np.random.default_rng(0)
    x = rng.standard_normal((B, CTOT, H, W), dtype=np.float32)
    cw = rng.uniform(-0.1, 0.1, (1, 6, 3, 3)).astype(np.float32)
    cb = np.array([0.01], dtype=np.float32)
    o = kernel(x, cw, cb)
    print(o.shape, o.dtype)


# revision 90
# speedup vs baseline: 1.0050x; 1.0050x over previous
"""Trainium2 Bass kernel for nn_CBAMSpaceMask (CBAM spatial mask over T timestep blocks).

Math per timestep block t (3 channels):
  mx_c = maxpool3x3(x_c)          (stride 1, -inf pad == replicate pad)
  av_c = avgpool3x3(x_c)/9        (zero pad, count_include_pad)
  y_t  = sum_c wM_c * mx_c + wA_c * av_c + b   (3x3 conv, zero pad)
  out[3t+c] = sigmoid(leakyrelu(y_t))          (broadcast over c)

Design (per core = 1 batch element, pure data parallel over batch):
  - host-side re-layout (pure layout, no compute): x4[g, r, j, c, w] =
    xpad[r+j, 6g+c, w] bf16 materializes the vertical-shift triplet
    contiguously per (group, row), so every main input load is ONE
    software-DGE trigger per group with fully-contiguous 18KB-per-partition
    runs on both sides (~35% faster DMA than fragmented reads); xt2 is the
    same idea for the small last-8-rows loads
  - groups of 6 planes (= 2 timesteps, one matmul pair); both row-subs
    (y rows 0:124 and 124:248) share a [128, 2sub, 3shift, 6, W] tile
  - pools: vertical 3-row max (2 DVE ops over the shift slices), horizontal
    3-tap max and box sum (2 DVE ops each) into 258-col zero-padded mx/bh;
    vertical box sum of the avg path folded into the conv operator (op@Bv);
    pad-column zeroing via ONE strided-AP memset per tile (cols 0 and 257)
  - conv: banded-Toeplitz matmuls on PE; timestep-paired rhs (planes
    {c, c+3} via stride-3 slice) -> all matmuls full-width N=512, padded
    windows supply the conv zero padding; psum [124, 2, 256]; max-path
    matmuls issued first so the bh pools may lag the mx pools
  - MOVED groups: the avg path runs on PE as 5 horizontal taps directly
    against X (effective kernel w_avg*[1,1,1], vertical box still in the
    band), with clipped rhs windows accumulating into column-offset psum
    slices (unwritten psum columns realize the conv zero pad) -> their bh
    DVE pools are skipped entirely. k=1 balances DVE (the bottleneck
    engine) against PE's late-phase wall.
  - last-8-rows chunk: rows 246..255 packed per quadrant at partition
    bases 0/32/64/96; kw taps folded into K=30 stacked matmuls using R/L
    column-shifted copies (shifts carry the zero edge in from memset pad
    cols); c2 pairs run at iterations 3..6, never in a low-p-state tail
  - epilogue: ACT Prelu(psum+bias, alpha=.01) IN PLACE on psum (same ACT
    table as sigmoid -> no table reloads) -> ACT Sigmoid -> bf16 sg tile.
    Output is ONE channel per timestep ([T, H, W]); the host broadcasts to
    the 3 channels (reference broadcasts before the elementwise sigmoid, so
    results are identical) -> output DMA volume and sigmoid work cut 3x
  - c2 sigmoids write a persistent [8, 16, W] tile; ONE final DMA stores
    rows 248..255 for all timesteps
  - startup: all early DMAs share the gpsimd software-DGE FIFO so issue
    order is true bandwidth priority (g0 per-sub, cst chunks split by
    first-use mat ranges, t2, g1, g2); group 0 is pooled per-sub with
    dedicated tiles so PE starts ~17us in; a burst of consumer-less
    keepalive matmuls into a scratch psum bank holds the PE p-state up
    through the one unavoidable pipeline-fill gap; loads run 3 groups
    ahead, pools 2 ahead of convs, output triggers age 2 checkpoints so
    no gpsimd trigger ever blocks on an incomplete producer
"""
import sys

sys.path.insert(0, "/opt/trn_rl_repo")

import numpy as np
import ml_dtypes
from contextlib import ExitStack

import concourse.bass as bass
import concourse.tile as tile
from concourse import bacc, mybir
from concourse.bass_utils import run_bass_kernel_spmd

F32 = mybir.dt.float32
BF16 = mybir.dt.bfloat16

B, CTOT, H, W = 8, 48, 256, 256
T = 16
N_CORES = 8
NGRP = 8            # groups of 6 planes = 2 timesteps
GP = 6              # planes per group
# main chunk geometry: y rows [m0,m1) from x rows [r0,r1)
SUBS = [(0, 124, 0, 128), (124, 248, 122, 250)]
C2 = (248, 256, 246, 256)   # last-8-rows chunk
NMAIN = 2 * 3 * 3 * 2       # path, c, kw, sub
NC2 = 2 * 3                 # path, c (kw folded into K=30)
NAVG5 = 3 * 5 * 2           # c, s, sub (avg path as 5-tap on X, no bh pools)
NCORR = 3 * 2 * 2           # c, side, sub (avg5 edge-column corrections)
NMAT = NMAIN + NC2 + NAVG5 + NCORR
# groups whose avg path runs on PE as 5 horizontal taps against X directly
# (skipping the bh DVE pools) — balances DVE (bottleneck) against PE slack
MOVED = (1,)
AVG5_TAPS = (-2, -1, 1, 2, 0)   # s=0 last: the stop matmul is full width
QP_T2 = 12                      # planes per t2 quadrant (2 groups)

_cache = {}


def _build_stack(conv_w):
    """lhsT stack [128, NMAT, 128] bf16.

    mats 0..35: main-sub ops, idx = ((path*3 + c)*3 + kw)*2 + sub,
      lhsT = op[m0:m1, r0:r1].T  ([K=128, M=124])
    mats 36..41: chunk-2 stacked ops, idx = 36 + path*3 + c,
      [K=30, M=8]: K blocks of 10 rows for kw = 1 (center), 0, 2,
      replicated at partition bases 0/32/64/96.
    """
    w = conv_w[0].astype(np.float64)  # [6, 3, 3]
    Bv = np.zeros((H, H))
    for i in (-1, 0, 1):
        Bv += np.eye(H, k=i)
    stack = np.zeros((128, NMAT, 128), dtype=np.float64)

    def band_op(path, c, kw):
        op = np.zeros((H, H))
        k2d = w[2 * c] if path == 0 else w[2 * c + 1]
        for kh in range(3):
            op += k2d[kh, kw] * np.eye(H, k=kh - 1)
        if path == 1:
            op = (op @ Bv) / 9.0
        return op

    def band_op5(c, s):
        """Avg path folded horizontally: 5-tap effective kernel e[s] =
        (w_avg[kh, :] * [1,1,1])[s]; vertical box stays in the band."""
        op = np.zeros((H, H))
        k2d = w[2 * c + 1]
        for kh in range(3):
            e = 0.0
            for kw in range(3):
                if abs(kw - 1 - s) <= 1:
                    e += k2d[kh, kw]
            op += e * np.eye(H, k=kh - 1)
        return (op @ Bv) / 9.0

    # mat layout is ordered by first use so the cst load can be split into
    # chunks positioned in the early DMA FIFO:
    #   [0:18]  sub0 main (max+avg3)   — first matmuls of group 0 sub 0
    #   [18:36] sub1 main
    #   [36:66] avg5 (MOVED groups)    — first used by group 1
    #   [66:72] c2                     — first used around iteration 3
    for path in range(2):
        for c in range(3):
            for kw in range(3):
                op = band_op(path, c, kw)
                for sub, (m0, m1, r0, r1) in enumerate(SUBS):
                    mat = _mat_main(path, c, kw, sub)
                    lhsT = op[m0:m1, r0:r1].T  # [K, M]
                    K, M = lhsT.shape
                    stack[:K, mat, :M] = lhsT
            # chunk 2: kw-stacked [30, 8]
            mat = _mat_c2(path, c)
            m0, m1, r0, r1 = C2
            for kwi, kw in enumerate((1, 0, 2)):
                lhsT = band_op(path, c, kw)[m0:m1, r0:r1].T  # [10, 8]
                for base in (0, 32, 64, 96):
                    stack[base + 10 * kwi:base + 10 * kwi + 10, mat, :8] = lhsT
    for c in range(3):
        for si, s in enumerate(AVG5_TAPS):
            op = band_op5(c, s)
            for sub, (m0, m1, r0, r1) in enumerate(SUBS):
                mat = _mat_avg5(c, si, sub)
                lhsT = op[m0:m1, r0:r1].T
                K, M = lhsT.shape
                stack[:K, mat, :M] = lhsT
    # avg5 edge corrections: the horizontal fold e = w * [1,1,1] is the
    # UNclipped composition, so at image cols 0/255 it spuriously includes
    # the kw=0/kw=2 weight against the edge x column (the reference zeroes
    # the whole out-of-range pooled value). Subtract that term.
    for c in range(3):
        for side, kw in ((0, 0), (1, 2)):
            op = np.zeros((H, H))
            for kh in range(3):
                op += w[2 * c + 1][kh, kw] * np.eye(H, k=kh - 1)
            op = -(op @ Bv) / 9.0
            for sub, (m0, m1, r0, r1) in enumerate(SUBS):
                lhsT = op[m0:m1, r0:r1].T
                K, M = lhsT.shape
                stack[:K, _mat_corr(c, side, sub), :M] = lhsT
    return stack.astype(ml_dtypes.bfloat16)


def _mat_main(path, c, kw, sub):
    return sub * 18 + (path * 3 + c) * 3 + kw


def _mat_avg5(c, si, sub):
    return 36 + sub * 15 + c * 5 + si


def _mat_c2(path, c):
    return 66 + path * 3 + c


def _mat_corr(c, side, sub):
    return 72 + (c * 2 + side) * 2 + sub


def _prep_x(xi):
    """Host-side re-layout.

    xp [row+pad, plane, w] bf16: row r holds image row r-1; rows 0 and 257
    replicate the image edge rows (the maxpool clamp; conv coefficients
    there are zero). Used by the small t2 (last-8-rows) loads.

    x4 [group, row, shift, plane-in-group, w] bf16: x4[g, r, j] = xp[r+j]
    for the 6 planes of group g. The shift triplet a partition needs for
    the vertical 3-max is CONTIGUOUS per (g, r), so every main input load
    is a fully-contiguous 18KB-per-partition HBM read (the DMA engines run
    ~35% faster on unfragmented source runs).
    """
    xp = np.empty((H + 2, CTOT, W), dtype=ml_dtypes.bfloat16)
    xp[1:H + 1] = xi.transpose(1, 0, 2)
    xp[0] = xp[1]
    xp[H + 1] = xp[H]
    v = np.lib.stride_tricks.as_strided(
        xp, shape=(NGRP, H, 3, GP, W),
        strides=(GP * W * 2, CTOT * W * 2, CTOT * W * 2, W * 2, 2))
    # t2 feed: xt2[q, j, r, c, w] = xp[246 + r + j, 12q + c, w] — contiguous
    # per (q, j, r) so the 12 small t2 loads are unfragmented
    r0 = C2[2]
    v2 = np.lib.stride_tricks.as_strided(
        xp[r0:], shape=(4, 3, 10, 12, W),
        strides=(12 * W * 2, CTOT * W * 2, CTOT * W * 2, W * 2, 2))
    return {"x4": np.ascontiguousarray(v), "xt2": np.ascontiguousarray(v2)}


def _build_program():
    nc = bacc.Bacc("TRN2", target_bir_lowering=False, debug=False, enable_asserts=False)
    x4_ap = nc.dram_tensor("x4", [NGRP, H, 3, GP, W], BF16,
                           kind="ExternalInput").ap()
    xt2_ap = nc.dram_tensor("xt2", [4, 3, 10, QP_T2, W], BF16,
                            kind="ExternalInput").ap()
    cst_ap = nc.dram_tensor("cst", [128, NMAT, 128], BF16, kind="ExternalInput").ap()
    bias_ap = nc.dram_tensor("bias", [128, 1], F32, kind="ExternalInput").ap()
    # bf16, one channel per timestep: sigmoid outputs lie in (0,1) so bf16
    # quantization (~0.4% rel) is far inside the accuracy budget; the host
    # upcasts to f32 and broadcasts each timestep mask to its 3 channels.
    out_ap = nc.dram_tensor("out", [T, H, W], BF16, kind="ExternalOutput").ap()

    MAXOP = mybir.AluOpType.max
    ADDOP = mybir.AluOpType.add
    RWST = CTOT * W          # HBM row stride (elements)

    with tile.TileContext(nc) as tc, ExitStack() as ctx:
        const_pool = ctx.enter_context(tc.tile_pool(name="const", bufs=1))
        psum_pool = ctx.enter_context(tc.tile_pool(name="psum", bufs=6, space="PSUM"))
        warm_pool = ctx.enter_context(tc.tile_pool(name="warm", bufs=1, space="PSUM"))
        sg_pool = ctx.enter_context(tc.tile_pool(name="sg", bufs=6))
        t2_pool = ctx.enter_context(tc.tile_pool(name="t2", bufs=1))
        # 4 x bufs: load_xud(g+3) at iteration g then recycles the buffer of
        # group g-1, whose conv readers (MOVED groups read X as the avg-path
        # rhs) were issued at iteration g-1 — program order stays consistent
        x_pool = ctx.enter_context(tc.tile_pool(name="xload", bufs=4))
        mxbh_pool = ctx.enter_context(tc.tile_pool(name="mxbh", bufs=3))
        g0_pool = ctx.enter_context(tc.tile_pool(name="g0", bufs=1))

        cst = const_pool.tile([128, NMAT, 128], BF16, tag="cst")
        bias = const_pool.tile([128, 1], F32, tag="bias")
        # (cst/bias DMAs are issued AFTER the first input loads: cst is only
        # needed by the first matmul ~13us in, while the input loads gate the
        # DVE pools — the 1.4MB cst transfer must not hog the DMA engines
        # during the first microseconds)

        # ---- t2 tiles: rows 246..255 of quadrant q (planes 12q..12q+11) at
        # partitions 32q..32q+9. MX/BH are padded to 258 cols (data at cols
        # 1..256, zero pads) and also hold R/L column-shifted copies at
        # partition offsets +10 / +20 (kw-folded K=30).
        WP = W + 2
        QP = 12  # planes per t2 quadrant (2 groups)
        T2X = t2_pool.tile([128, QP, W], BF16, tag="t2x")
        T2U = t2_pool.tile([128, QP, W], BF16, tag="t2u")
        T2D = t2_pool.tile([128, QP, W], BF16, tag="t2d")
        T2MX = t2_pool.tile([128, QP, WP], BF16, tag="t2mx")
        T2BH = t2_pool.tile([128, QP, WP], BF16, tag="t2bh")
        # c2 sigmoid accumulator: rows 248..255 x all 16 timesteps; ONE
        # final DMA stores it
        C2OUT = t2_pool.tile([8, T, W], BF16, tag="c2out")
        # (no full-tile zeroing: garbage in gap partitions only flows into
        # regions later overwritten by the shift DMAs or never read; the
        # pad columns that ARE read get strided memsets in t2_pools)

        # ---- fused input load: tile [128, 2 sub, 3 shift, GP, W]; shift j
        # holds padded rows (r0_sub + p + j) so the vertical 3-max is three
        # aligned slices of ONE tile. The host-materialized x4 layout makes
        # the triplet contiguous per (group, row): ONE trigger per group,
        # fully-contiguous 18KB packets on both sides.
        xud_tiles = {}
        RB = 3 * GP * W  # x4 row block (one partition's triplet), elements

        def _src_ap(g, subs=(0, 1)):
            dims = [[RB, 128]]
            if len(subs) == 2:
                dims.append([SUBS[1][2] * RB, 2])
            dims += [[1, RB]]
            off = g * H * RB + SUBS[subs[0]][2] * RB
            return bass.AP(x4_ap.tensor, off, dims)

        def load_xud(g):
            X = x_pool.tile([128, 2, 3, GP, W], BF16, tag="x")
            nc.gpsimd.dma_start(out=X[:], in_=_src_ap(g))
            xud_tiles[g] = X
            return X

        def load_xc(g):
            """Center slice only (both subs) — feeds a MOVED group's avg5
            matmuls, which can then run before the U/D halves even arrive."""
            X = x_pool.tile([128, 2, 3, GP, W], BF16, tag="x")
            dims = [[RB, 128], [SUBS[1][2] * RB, 2], [1, GP * W]]
            nc.gpsimd.dma_start(out=X[:, :, 1:2],
                                in_=bass.AP(x4_ap.tensor,
                                            g * H * RB + GP * W, dims))
            xud_tiles[g] = X
            return X

        def load_ud(g):
            X = xud_tiles[g]
            for sub in range(2):
                off = g * H * RB + SUBS[sub][2] * RB
                dims = [[RB, 128], [2 * GP * W, 2], [1, GP * W]]
                nc.gpsimd.dma_start(out=X[:, sub, 0:3:2],
                                    in_=bass.AP(x4_ap.tensor, off, dims))

        def pools(g, X=None, mx=None, bh=None, sub=None):
            """DVE pools; when sub is given, operate on that sub slice only
            (used for group 0's fast start with dedicated tiles). Groups in
            MOVED skip the bh pools (their avg path runs on PE against X)."""
            moved = g in MOVED
            if X is None:
                X = xud_tiles.pop(g)
            if mx is None:
                mx = mxbh_pool.tile([128, 2, GP, WP], BF16, tag="mx")
                bh = None if moved else mxbh_pool.tile([128, 2, GP, WP], BF16,
                                                       tag="bh")
            s = slice(None) if sub is None else slice(sub, sub + 1)
            D, XC, U = X[:, s, 0], X[:, s, 1], X[:, s, 2]
            # one strided memset zeroes both pad columns (0 and 257)
            nc.vector.memset(mx[:, s, :, 0:258:257], 0)
            # vertical 3-row max (DVE), in place into the U slice
            vx = U
            nc.vector.tensor_tensor(out=vx, in0=U, in1=D, op=MAXOP)
            nc.vector.tensor_tensor(out=vx, in0=vx, in1=XC, op=MAXOP)
            # horizontal 3-tap max (DVE) into padded mx
            nc.vector.tensor_tensor(out=mx[:, s, :, 1:256], in0=vx[:, :, :, 0:255],
                                    in1=vx[:, :, :, 1:256], op=MAXOP)
            nc.vector.tensor_copy(mx[:, s, :, 256:257], vx[:, :, :, 255:256])
            nc.vector.tensor_tensor(out=mx[:, s, :, 2:257], in0=mx[:, s, :, 2:257],
                                    in1=vx[:, :, :, 0:255], op=MAXOP)
            if not moved:
                # horizontal 3-tap box sum (DVE) into padded bh
                nc.vector.memset(bh[:, s, :, 0:258:257], 0)
                nc.vector.tensor_tensor(out=bh[:, s, :, 1:256],
                                        in0=XC[:, :, :, 0:255],
                                        in1=XC[:, :, :, 1:256], op=ADDOP)
                nc.vector.tensor_copy(bh[:, s, :, 256:257], XC[:, :, :, 255:256])
                nc.vector.tensor_tensor(out=bh[:, s, :, 2:257],
                                        in0=bh[:, s, :, 2:257],
                                        in1=XC[:, :, :, 0:255], op=ADDOP)
            return mx, bh, X

        def load_t2(q):
            # gpsimd FIFO (so these cannot steal engine bandwidth from the
            # earlier critical loads); the xt2 layout keeps them contiguous
            b = 32 * q
            nc.gpsimd.dma_start(out=T2X[b:b + 10], in_=xt2_ap[q, 1])
            nc.gpsimd.dma_start(out=T2U[b:b + 10], in_=xt2_ap[q, 2])
            nc.gpsimd.dma_start(out=T2D[b:b + 10], in_=xt2_ap[q, 0])

        def _t2_shifts(tl):
            # R/L column-shifted copies into partition blocks +10 / +20.
            # All matmul rhs windows read cols 1..256 of their block:
            #   block +10 pairs kw=0 (needs P[w-1]): dest col j <- data col j-1
            #   block +20 pairs kw=2 (needs P[w+1]): dest col j <- data col j+1
            # The widened [0:256]/[2:258] sources carry the zero pad edge.
            for q in range(4):
                b = 32 * q
                nc.sync.dma_start(out=tl[b + 10:b + 20, :, 1:257],
                                  in_=tl[b:b + 10, :, 0:256])
                nc.sync.dma_start(out=tl[b + 20:b + 30, :, 1:257],
                                  in_=tl[b:b + 10, :, 2:258])

        def t2_pools(chunk):
            """Pools over the packed t2 tile, issued in 3 chunks so the DVE
            bursts interleave between main-group pools instead of stalling
            a whole group's worth of PE work."""
            vx = T2U
            if chunk == 0:
                nc.vector.tensor_tensor(out=vx[:], in0=T2U[:], in1=T2D[:],
                                        op=MAXOP)
                nc.vector.tensor_tensor(out=vx[:], in0=vx[:], in1=T2X[:],
                                        op=MAXOP)
                nc.vector.memset(T2MX[:, :, 0:258:257], 0)
                nc.vector.memset(T2BH[:, :, 0:258:257], 0)
            elif chunk == 1:
                nc.vector.tensor_tensor(out=T2MX[:, :, 1:256], in0=vx[:, :, 0:255],
                                        in1=vx[:, :, 1:256], op=MAXOP)
                nc.vector.tensor_copy(T2MX[:, :, 256:257], vx[:, :, 255:256])
                nc.vector.tensor_tensor(out=T2MX[:, :, 2:257],
                                        in0=T2MX[:, :, 2:257],
                                        in1=vx[:, :, 0:255], op=MAXOP)
                _t2_shifts(T2MX)
            else:
                nc.vector.tensor_tensor(out=T2BH[:, :, 1:256], in0=T2X[:, :, 0:255],
                                        in1=T2X[:, :, 1:256], op=ADDOP)
                nc.vector.tensor_copy(T2BH[:, :, 256:257], T2X[:, :, 255:256])
                nc.vector.tensor_tensor(out=T2BH[:, :, 2:257],
                                        in0=T2BH[:, :, 2:257],
                                        in1=T2X[:, :, 0:255], op=ADDOP)
                _t2_shifts(T2BH)

        out_ready = []   # sigmoids surely complete: safe to issue triggers
        out_recent = []  # freshly issued sigmoids: age one checkpoint first

        def epilogue_lrelu(ps, M):
            """ACT Prelu(psum + bias) IN PLACE on the psum bank. Prelu
            (parametric_relu) lives in the same ACT function table as
            sigmoid, so alternating them costs no ACT_TABLE_LOADs."""
            nc.scalar.activation(ps[0:M], ps[0:M],
                                 mybir.ActivationFunctionType.Prelu,
                                 bias=bias[0:M], scale=1.0, alpha=0.01)
            return ps

        def epilogue_sigmoid(ps, M, t0, m0, m1):
            """Sigmoid psum -> bf16 sg (one channel per timestep). Output
            DMA issue is DEFERRED (gpsimd queue is in-order: a trigger
            waiting on its sigmoid would stall later load triggers)."""
            sg = sg_pool.tile([128, 2, W], BF16, tag="epis")
            nc.scalar.activation(sg[0:M], ps[0:M],
                                 mybir.ActivationFunctionType.Sigmoid)
            out_recent.append((sg, M, t0, m0, m1))

        def flush_outputs(final=False):
            for sg, M, t0, m0, m1 in out_ready:
                dst = out_ap[t0:t0 + 2, m0:m1, :].transpose([1, 0, 2])
                nc.gpsimd.dma_start(out=dst, in_=sg[0:M])
            out_ready.clear()
            out_ready.extend(out_recent)
            out_recent.clear()
            if final and out_ready:
                flush_outputs()

        def conv_sub(g, sub, mx, bh, X):
            """Main-chunk accumulation for group g's timestep pair.

            Max-path matmuls are full-width N=512 against the padded mx
            (whose zero pads supply the conv zero padding). For MOVED
            groups the avg path is 5 horizontal taps against X directly:
            clipped rhs windows accumulate into column-offset psum slices,
            so unwritten psum columns realize the conv zero padding.
            """
            m0, m1, r0, r1 = SUBS[sub]
            M, K = m1 - m0, r1 - r0
            moved = g in MOVED
            sb = 0 if mx.shape[1] == 1 else sub
            ps = psum_pool.tile([128, 2, W], F32, tag="ps")
            i, n = 0, 30 if moved else 18
            for c in range(3):
                for kw in (1, 0, 2):
                    s = kw - 1
                    mat = _mat_main(0, c, kw, sub)
                    rhs = mx[0:K, sb, c:c + 4:3, 1 + s:257 + s]
                    nc.tensor.matmul(ps[0:M], cst[0:K, mat, 0:M], rhs,
                                     start=(i == 0), stop=(i == n - 1))
                    i += 1
            if moved:
                XC = X[:, sb, 1]
                for si, s in enumerate(AVG5_TAPS):
                    a, b = max(0, s), W + min(0, s)
                    for c in range(3):
                        mat = _mat_avg5(c, si, sub)
                        rhs = XC[0:K, c:c + 4:3, a:b]
                        nc.tensor.matmul(ps[0:M, :, a - s:b - s],
                                         cst[0:K, mat, 0:M], rhs,
                                         start=False, stop=(i == n - 1))
                        i += 1
                # edge-column corrections (N=2 each): subtract the spurious
                # kw=0 / kw=2 term at image cols 0 / 255
                for side, col in ((0, 0), (1, W - 1)):
                    for c in range(3):
                        rhs = XC[0:K, c:c + 4:3, col:col + 1]
                        nc.tensor.matmul(ps[0:M, :, col:col + 1],
                                         cst[0:K, _mat_corr(c, side, sub), 0:M],
                                         rhs, start=False, stop=(i == n - 1))
                        i += 1
            else:
                for c in range(3):
                    for kw in (1, 0, 2):
                        s = kw - 1
                        mat = _mat_main(1, c, kw, sub)
                        rhs = bh[0:K, sb, c:c + 4:3, 1 + s:257 + s]
                        nc.tensor.matmul(ps[0:M], cst[0:K, mat, 0:M], rhs,
                                         start=False, stop=(i == n - 1))
                        i += 1
            epilogue_lrelu(ps, M)
            return ps, M, 2 * g, m0, m1

        def conv_avg5(g, sub, X):
            """Open a MOVED group's psum with its 15 avg5 taps (needs only
            the XC slice + cst chunk C — runs while U/D are still loading).
            The full-width s=0 tap goes first to initialize the psum."""
            m0, m1, r0, r1 = SUBS[sub]
            M, K = m1 - m0, r1 - r0
            ps = psum_pool.tile([128, 2, W], F32, tag="ps")
            XC = X[:, sub, 1]
            first = True
            for si, s in ((4, 0), (0, -2), (1, -1), (2, 1), (3, 2)):
                a, b = max(0, s), W + min(0, s)
                for c in range(3):
                    rhs = XC[0:K, c:c + 4:3, a:b]
                    nc.tensor.matmul(ps[0:M, :, a - s:b - s],
                                     cst[0:K, _mat_avg5(c, si, sub), 0:M],
                                     rhs, start=first, stop=False)
                    first = False
            return ps

        def conv_max(g, sub, mx, ps):
            """Close the psum with the 9 max-path matmuls."""
            m0, m1, r0, r1 = SUBS[sub]
            M, K = m1 - m0, r1 - r0
            i = 0
            for c in range(3):
                for kw in (1, 0, 2):
                    s = kw - 1
                    rhs = mx[0:K, sub, c:c + 4:3, 1 + s:257 + s]
                    nc.tensor.matmul(ps[0:M], cst[0:K, _mat_main(0, c, kw, sub), 0:M],
                                     rhs, start=False, stop=(i == 8))
                    i += 1
            epilogue_lrelu(ps, M)
            return ps, M, 2 * g, m0, m1

        def conv_c2(g):
            """Last-8-rows accumulation (kw-folded, K=30) for group g; the
            sigmoid lands in the persistent C2OUT tile."""
            m0, m1, r0, r1 = C2
            M = m1 - m0
            b = 32 * (g // 2)
            pb = 6 * (g % 2)
            ps = psum_pool.tile([128, 2, W], F32, tag="ps")
            idx = 0
            for path in range(2):
                for c in range(3):
                    mat = _mat_c2(path, c)
                    src = T2MX if path == 0 else T2BH
                    rhs = src[b:b + 30, pb + c:pb + c + 4:3, 1:257]
                    nc.tensor.matmul(ps[0:M], cst[b:b + 30, mat, 0:M], rhs,
                                     start=(idx == 0), stop=(idx == NC2 - 1),
                                     tile_position=(b, 0))
                    idx += 1
            epilogue_lrelu(ps, M)
            nc.scalar.activation(C2OUT[0:M, 2 * g:2 * g + 2], ps[0:M],
                                 mybir.ActivationFunctionType.Sigmoid)

        # ---- schedule: group 0 is loaded per-sub with dedicated pool tiles
        # so the first matmul only waits on sub 0's load + 6 DVE ops. Later
        # loads run 3 groups ahead; pools for g+2 are issued at the top of
        # iteration g so they execute while PE runs group g's convs. Output
        # triggers age through two checkpoints before issue.
        # conv_c2(g) is deferred three iterations (it only needs the t2 pools
        # and a psum bank), so t2 pool work stays off the early critical path
        # ---- early DMA FIFO (all on the gpsimd queue, so issue order is
        # bandwidth priority): g0 per-sub first (sub0 gates everything),
        # then the cst chunks in first-use order, t2, g1, g2.
        g0X = x_pool.tile([128, 2, 3, GP, W], BF16, tag="x")
        for sub in range(2):
            nc.gpsimd.dma_start(out=g0X[:, sub:sub + 1], in_=_src_ap(0, (sub,)))
        nc.gpsimd.dma_start(out=cst[:, 0:18], in_=cst_ap[:, 0:18, :])
        nc.gpsimd.dma_start(out=cst[:, 18:36], in_=cst_ap[:, 18:36, :])
        for q in range(4):
            load_t2(q)
        load_xud(1)
        nc.gpsimd.dma_start(out=cst[:, 36:66], in_=cst_ap[:, 36:66, :])
        load_xud(2)
        nc.gpsimd.dma_start(out=cst[:, 66:84], in_=cst_ap[:, 66:84, :])
        nc.gpsimd.dma_start(out=bias[:], in_=bias_ap)
        # DVE: g0 pools (split per sub for the earliest first matmul), then
        # the t2 chunks woven between p1/p2
        g0t = []
        for sub in range(2):
            mxs = g0_pool.tile([128, 1, GP, WP], BF16, tag=f"g0mx{sub}")
            bhs = (None if 0 in MOVED else
                   g0_pool.tile([128, 1, GP, WP], BF16, tag=f"g0bh{sub}"))
            mxs, bhs, _ = pools(0, X=g0X[:, sub:sub + 1], mx=mxs, bh=bhs, sub=0)
            g0t.append((mxs, bhs, g0X[:, sub:sub + 1]))
        t2_pools(0)
        pools_of = {1: pools(1)}
        t2_pools(1)
        pools_of[2] = pools(2)
        t2_pools(2)
        # c2 pairs run at iterations 3..6, issued BEFORE the group's convs:
        # they fill the PE stall windows where pools lag, and none are left
        # for a low-p-state tail
        for g in range(NGRP):
            if g + 3 < NGRP:
                load_xud(g + 3)
            if g + 2 < NGRP and g >= 1:
                pools_of[g + 2] = pools(g + 2)
            if 3 <= g <= 6:
                conv_c2(2 * (g - 3))
                conv_c2(2 * (g - 3) + 1)
            if g == 0:
                epilogue_sigmoid(*conv_sub(0, 0, *g0t[0]))
                epilogue_sigmoid(*conv_sub(0, 1, *g0t[1]))
                # PE keepalive: group 1's matmuls cannot start until its
                # pools clear the t2 chunks (~18us idle). Idle drops the PE
                # to a low p-state and the first ~20 real matmuls after the
                # gap run ~2x slow. Dummy matmuls into a scratch psum bank
                # (no consumers) hold the clock up through the gap.
                scratch = warm_pool.tile([128, 2, W], F32, tag="warm")
                for i in range(32):
                    nc.tensor.matmul(scratch[0:124], cst[0:128, 0, 0:124],
                                     g0t[0][0][0:128, 0, 0:4:3, 1:257],
                                     start=(i == 0), stop=(i == 31))
            else:
                mx, bh, X = pools_of.pop(g)
                epilogue_sigmoid(*conv_sub(g, 0, mx, bh, X))
                epilogue_sigmoid(*conv_sub(g, 1, mx, bh, X))
            flush_outputs()
        flush_outputs(final=True)
        # one DMA for all last-8 rows: [T, 8, W] <- C2OUT[0:8] transposed
        m0 = C2[0]
        nc.sync.dma_start(out=out_ap[:, m0:m0 + 8, :].transpose([1, 0, 2]),
                          in_=C2OUT[0:8])

    nc.compile()
    return nc


def kernel(input_tensor, conv_w, conv_b):
    input_tensor = np.ascontiguousarray(np.asarray(input_tensor, dtype=np.float32))
    conv_w = np.asarray(conv_w, dtype=np.float32)
    conv_b = np.asarray(conv_b, dtype=np.float32)

    if "nc" not in _cache:
        _cache["nc"] = _build_program()
    nc = _cache["nc"]

    stack = _build_stack(conv_w)
    bias_vec = np.full((128, 1), conv_b[0], dtype=np.float32)
    in_maps = [
        {**_prep_x(input_tensor[i]), "cst": stack, "bias": bias_vec}
        for i in range(N_CORES)
    ]
    res = run_bass_kernel_spmd(nc, in_maps, list(range(N_CORES)))
    # [T, H, W] bf16 per core -> broadcast each timestep mask to 3 channels
    out = np.stack([res.results[i]["out"] for i in range(N_CORES)], axis=0)
    out = np.repeat(out.astype(np.float32), 3, axis=1)
    return out


if __name__ == "__main__":
    rng = np.random.default_rng(0)
    x = rng.standard_normal((B, CTOT, H, W), dtype=np.float32)
    cw = rng.uniform(-0.1, 0.1, (1, 6, 3, 3)).astype(np.float32)
    cb = np.array([0.01], dtype=np.float32)
    o = kernel(x, cw, cb)
    print(o.shape, o.dtype)
